# revision 26
# baseline (speedup 1.0000x reference)
"""BiTreeLSTM forward pass on 8 TRN2 NeuronCores.

Strategy (8-way tensor parallel on the hidden/gate dimension):
  - Core k owns hidden dims [128k, 128k+128) -> a 768-row slice of the 6144
    gate rows (6 gate types x 128 dims) plus matching slices of c/h/px.
  - Input projections x_gates = feat @ Wx.T and px = feat @ Wpx.T are computed
    on-device as a big bf16 GEMM, split across cores by output columns.  All
    biases are folded into the GEMM through a 128-row "ones feature" block.
    Gate rows are pre-scaled by 64 host-side so the recurrence fp8 weights
    (also x64) accumulate in a consistently scaled PSUM; activations apply
    scale=1/64.
  - The leaf-to-root recurrence is batched by tree level.  Per level:
    8 DoubleRow fp8 matmuls per gate chunk (256-wide K each) accumulate
    Wlr.T @ [h_l | h_r] into PSUM on top of an identity matmul that seeded
    PSUM with the x_gates slice (issued before the AllGather lands, so the
    PE has work during the collective), ScalarE applies sigmoid/tanh with
    scale=1/64 into bf16 gate tiles, VectorE runs the c/h elementwise chain
    in bf16, and an 8-core AllGather of the fp8 h slices replicates the new
    h into every core's fp8 h table (half the payload of bf16).
  - h lives ONLY in fp8 (matmul operand + AG payload); c is kept in bf16
    per-core slices; the fp32 output rows are written off the critical path.
  - A dummy 8-byte AllGather is issued at program start to absorb core
    launch skew off the critical path.
"""

import sys

import numpy as np

_REPO = "/opt/trn_rl_repo"
if _REPO not in sys.path:
    sys.path.insert(0, _REPO)

import ml_dtypes  # noqa: E402

import concourse.bass as bass  # noqa: E402,F401
import concourse.mybir as mybir  # noqa: E402
import concourse.tile as tile  # noqa: E402
from concourse import bacc  # noqa: E402
from concourse.bass_utils import run_bass_kernel_spmd  # noqa: E402

NCORES = 8
H = 1024
HS = H // NCORES          # 128 hidden dims per core
S6 = 6 * HS               # 768 gate rows per core
HC = H // 128             # 8 h-table chunks
KC = 2 * H // 128         # 16 recurrence K chunks (8 DoubleRow pairs)
SEG = 512                 # max nodes per psum segment
FTW = 256                 # feat tile width (node columns per GEMM tile)
BIAS_ROWS = 128           # ones-feature rows used to fold biases into the GEMM
GSC = 64.0                # gate pre-scale folded into weights (fp8 range)

AF = mybir.ActivationFunctionType
ALU = mybir.AluOpType
PM = mybir.MatmulPerfMode
BF16 = mybir.dt.bfloat16
F32 = mybir.dt.float32
FP8 = mybir.dt.float8e4
BF16_NP = ml_dtypes.bfloat16
FP8_NP = ml_dtypes.float8_e4m3

# per-core gate chunk order: i, o, f_l, f_r, r, u  (u = tanh is last so the
# five sigmoid chunks are contiguous); original stack order is i,o,fl,fr,u,r
GATE_PERM = [0, 1, 2, 3, 5, 4]
GATE_FUNCS = [AF.Sigmoid] * 5 + [AF.Tanh]
G_I, G_O, G_FL, G_FR, G_R, G_U = range(6)

USE_DR = True      # DoubleRow fp8 matmuls (debug flag)
H8 = True          # fp8 h table / AG payload / wlr weights (debug flag)
DBG = False        # emit debug dumps (sim only)
DUMMY_AG = False   # startup-skew absorbing dummy collective


def _hdt():
    return (mybir.dt.float8e4 if H8 else mybir.dt.bfloat16,
            ml_dtypes.float8_e4m3 if H8 else ml_dtypes.bfloat16)


# ---------------------------------------------------------------- schedule --

def _runs(vals, limit, region=None):
    """Decompose an int list into (start, step, count, pos) arithmetic runs.

    A run is kept only if [start, start+step*count) stays within `limit` and
    step >= 1; otherwise singletons.  With `region`, runs additionally never
    cross a multiple-of-region boundary (needed for region-tiled tables).
    """
    out = []
    i, m = 0, len(vals)
    while i < m:
        j = i + 1
        if j < m:
            step = vals[j] - vals[i]
            while j + 1 < m and vals[j + 1] - vals[j] == step:
                if region is not None and vals[j + 1] // region != vals[i] // region:
                    break
                j += 1
            if region is not None and vals[j] // region != vals[i] // region:
                # walk back to stay inside the region
                while j > i and vals[j] // region != vals[i] // region:
                    j -= 1
            cnt = j - i + 1
            lim = limit
            if region is not None:
                lim = min(lim, (vals[i] // region + 1) * region)
            if cnt > 1 and step >= 1 and vals[i] + step * cnt <= lim:
                out.append((vals[i], step, cnt, i))
                i = j + 1
                continue
        out.append((vals[i], 1, 1, i))
        i += 1
    return out


def _schedule(left, right, n):
    """Levelize the tree exactly matching the reference scan semantics.

    Reference processes i = n-1 .. 0; h_all[l] reads the computed value iff
    l > i, else the initial zero.  Remapping l<=i (or out of range) to the
    zero sentinel `n` makes all deps point to higher indices, so grouping by
    longest-path level gives a valid batched schedule.
    """
    idx = np.arange(n)
    l = np.asarray(left).astype(np.int64)
    r = np.asarray(right).astype(np.int64)
    l = np.where((l > idx) & (l >= 0) & (l <= n), l, n)
    r = np.where((r > idx) & (r >= 0) & (r <= n), r, n)
    lev = np.empty(n + 1, np.int64)
    lev[n] = -1
    for i in range(n - 1, -1, -1):
        lev[i] = 1 + max(lev[l[i]], lev[r[i]])
    pad = n + 4
    levels = []
    for v in range(int(lev[:n].max()) + 1):
        nodes = np.where(lev[:n] == v)[0].tolist()
        segs = []
        for s0 in range(0, len(nodes), SEG):
            seg = nodes[s0:s0 + SEG]
            segs.append(dict(
                bs=len(seg),
                off=s0,
                node_runs=_runs(seg, n, region=FTW),
                l_runs=_runs([int(l[i]) for i in seg], pad),
                r_runs=_runs([int(r[i]) for i in seg], pad),
            ))
        levels.append(dict(B=len(nodes), nodes=nodes, segs=segs))
    return levels


def _feat_tiles(levels, n):
    """FTW-wide node-column tiles ordered by the first level that needs them."""
    minlev = np.full(n, 1 << 30, np.int64)
    for v, L in enumerate(levels):
        minlev[L["nodes"]] = v
    tiles = []
    for c0 in range(0, n, FTW):
        w = min(FTW, n - c0)
        tiles.append((c0, w, int(minlev[c0:c0 + w].min())))
    tiles.sort(key=lambda t: (t[2], t[0]))
    return tiles


# ----------------------------------------------------------------- builder --

def _c3(ap2, a, cnt, step=1):
    """3D column view [P, cnt, 1] of ap2[:, a : a+step*cnt : step]."""
    if cnt == 1 or step == 1:
        return ap2[:, a:a + cnt].rearrange("p (k s) -> p k s", s=1)
    return ap2[:, a:a + step * cnt].rearrange("p (k s) -> p k s", s=step)[:, :, 0:1]


def _c4(ap3, c0, nch, a, cnt, step=1):
    """4D view [P, nch, cnt, 1] of ap3[:, c0:c0+nch, a : a+step*cnt : step]."""
    if cnt == 1 or step == 1:
        return ap3[:, c0:c0 + nch, a:a + cnt].rearrange(
            "p c (k s) -> p c k s", s=1)
    return ap3[:, c0:c0 + nch, a:a + step * cnt].rearrange(
        "p c (k s) -> p c k s", s=step)[:, :, :, 0:1]


def build(nc, levels, feat_tiles, n, f):
    fc = f // 128
    fa = fc + 1               # + bias chunk
    pad = n + 4
    nlev = len(levels)
    max_b = max(L["B"] for L in levels)
    nreg = (n + FTW - 1) // FTW

    featT = nc.dram_tensor("featT", [f + BIAS_ROWS, n], BF16, kind="ExternalInput")
    hdt, _ = _hdt()
    if USE_DR == 2:
        wlrT = nc.dram_tensor("wlrT", [128, (KC // 2) * 2 * S6], hdt,
                              kind="ExternalInput")
    else:
        wlrT = nc.dram_tensor("wlrT", [2 * H, S6], hdt, kind="ExternalInput")
    wxpxT = nc.dram_tensor("wxpxT", [f + BIAS_ROWS, S6 + HS], BF16,
                           kind="ExternalInput")
    ident = nc.dram_tensor("ident", [128, 128], BF16, kind="ExternalInput")
    out = nc.dram_tensor("out", [HS, n], F32, kind="ExternalOutput")

    with tile.TileContext(nc) as tc:
        with (
            tc.tile_pool(name="wp", bufs=1) as wp,
            tc.tile_pool(name="tp", bufs=1) as tp,
            tc.tile_pool(name="fp", bufs=2) as fp,
            tc.tile_pool(name="ep", bufs=2) as ep,
            tc.tile_pool(name="sp", bufs=2) as sp,
            tc.tile_pool(name="pg", bufs=2, space="PSUM") as pgp,
            tc.tile_pool(name="pr", bufs=1, space="PSUM") as prp,
            tc.tile_pool(name="dp", bufs=4, space="DRAM") as dp,
        ):
            # ---- persistent SBUF ----
            wlr_sb = wp.tile([128, KC * S6], hdt, name="wlr_sb")
            wlri_v = wlr_sb.rearrange("p (c x) -> p c x", c=KC // 2)
            wx_sb = wp.tile([128, fa * (S6 + HS)], BF16, name="wx_sb")
            ident_sb = wp.tile([128, 128], BF16, name="ident_sb")

            hT = tp.tile([128, HC * pad], hdt, name="hT")
            cT = tp.tile([HS, pad], BF16, name="cT")
            houtT = tp.tile([HS, n], F32, name="houtT")
            xgr = [tp.tile([HS, 6 * FTW], BF16, name=f"xgr{i}")
                   for i in range(nreg)]
            pxr = [tp.tile([HS, FTW], BF16, name=f"pxr{i}")
                   for i in range(nreg)]
            stageT = tp.tile([HS, max_b], hdt, name="stageT")

            wlr_v = wlr_sb.rearrange("p (c x) -> p c x", c=KC)
            wx_v = wx_sb.rearrange("p (c x) -> p c x", c=fa)
            hT_v = hT.rearrange("p (c x) -> p c x", c=HC)

            # ---- startup-skew absorber: tiny collective at program start
            # (input bounced straight from a DRAM input so the SP queue can
            # issue it before any compute dependency forms)
            if DUMMY_AG:
                dagi = dp.tile([HS, 32], BF16, tag="dagi", name="dagi")
                dago = dp.tile([H, 32], BF16, tag="dago", name="dago",
                               addr_space="Shared")
                nc.sync.dma_start(out=dagi[:, :], in_=ident[:, 0:32])
                nc.gpsimd.collective_compute(
                    "AllGather", ALU.bypass,
                    replica_groups=[list(range(NCORES))],
                    ins=[dagi.opt()], outs=[dago.opt()])

            # ---- loads + zero sentinels ----
            if USE_DR == 2:
                nc.sync.dma_start(out=wlr_sb[:, :], in_=wlrT[:, :])
            else:
                nc.sync.dma_start(
                    out=wlr_v[:, :, :],
                    in_=wlrT.rearrange("(c p) x -> p c x", p=128))
            nc.sync.dma_start(
                out=wx_v[:, :, :],
                in_=wxpxT.rearrange("(c p) x -> p c x", p=128))
            nc.sync.dma_start(out=ident_sb[:, :], in_=ident[:, :])
            for c in range(HC):
                nc.vector.memset(hT_v[:, c, n:n + 1], 0.0)
            nc.vector.memset(cT[:, n:n + 1], 0.0)
            nc.vector.memset(stageT[:, :], 0.0)

            # ---- GEMM for one feat tile (node cols c0..c0+w) ----
            def emit_gemm_tile(c0, w):
                reg = c0 // FTW
                ft = fp.tile([128, fa * FTW], BF16, tag="ft", name=f"ft_{c0}")
                ft_v = ft.rearrange("p (c x) -> p c x", c=fa)
                nc.sync.dma_start(
                    out=ft_v[:, :, :w],
                    in_=featT.rearrange("(c p) x -> p c x", p=128)[:, :, c0:c0 + w])
                for m in range(7):
                    ps = pgp.tile([128, FTW], F32, tag="pg", name=f"pg_{c0}_{m}")
                    for c in range(fa):
                        nc.tensor.matmul(
                            ps[:, :w],
                            lhsT=wx_v[:, c, m * 128:(m + 1) * 128],
                            rhs=ft_v[:, c, :w],
                            start=(c == 0), stop=(c == fa - 1))
                    if m < 6:
                        nc.vector.tensor_copy(
                            out=xgr[reg][:, m * FTW:m * FTW + w], in_=ps[:, :w])
                    else:
                        nc.vector.tensor_copy(
                            out=pxr[reg][:, :w], in_=ps[:, :w])

            # ---- one recurrence level ----
            def emit_level(li):
                L = levels[li]
                B = L["B"]
                stage = stageT
                for seg in L["segs"]:
                    bs, off = seg["bs"], seg["off"]
                    node_runs = seg["node_runs"]

                    g = ep.tile([HS, 6 * SEG], BF16, tag="g", name=f"g_{li}")
                    if li == 0:
                        # leaves: gates come straight from x_gates (bias
                        # already folded in); no matmuls at all.
                        for m in range(6):
                            for (a, st, cnt, pos) in node_runs:
                                reg, loc = a // FTW, a % FTW
                                nc.scalar.activation(
                                    _c3(g, m * bs + pos, cnt),
                                    _c3(xgr[reg], m * FTW + loc, cnt, st),
                                    GATE_FUNCS[m], scale=1.0 / GSC)
                    else:
                        pt = prp.tile([128, 6 * SEG], F32, tag="pr",
                                      name=f"pr_{li}")

                        # seed PSUM with x_gates via identity matmul first:
                        # the PE does this during the previous AllGather.
                        # start=True zeroes the whole 2KB psum bank, so only
                        # the first write into each bank may carry it.
                        banks_seen = set()
                        for m in range(6):
                            for (a, st, cnt, pos) in node_runs:
                                reg, loc = a // FTW, a % FTW
                                bank = (m * bs + pos) * 4 // 2048
                                nc.tensor.matmul(
                                    pt[:, m * bs + pos:m * bs + pos + cnt],
                                    lhsT=ident_sb[:, :],
                                    rhs=_c3(xgr[reg], m * FTW + loc, cnt, st),
                                    start=bank not in banks_seen, stop=False,
                                    skip_group_check=True)
                                banks_seen.add(bank)
                        # DoubleRow fp8 child contributions (K pairs).
                        if USE_DR == 2:
                            for m in range(6):
                                for p in range(KC // 2):
                                    runs = seg["l_runs"] if p < HC // 2 else seg["r_runs"]
                                    last = p == KC // 2 - 1
                                    lhsT = wlri_v[
                                        :, p, m * 256:(m + 1) * 256].rearrange(
                                        "p (c x) -> p c x", c=2)
                                    for (a, st, cnt, pos) in runs:
                                        ch = (2 * p) % HC
                                        nc.tensor.matmul(
                                            pt[:, m * bs + pos:m * bs + pos + cnt],
                                            lhsT=lhsT,
                                            rhs=_c4(hT_v, ch, 2, a, cnt, st),
                                            start=False, stop=last,
                                            perf_mode=PM.DoubleRowSwInterleave,
                                            skip_group_check=True)
                        elif USE_DR:
                            for m in range(6):
                                for p in range(KC // 2):
                                    runs = seg["l_runs"] if p < HC // 2 else seg["r_runs"]
                                    last = p == KC // 2 - 1
                                    for (a, st, cnt, pos) in runs:
                                        ch = (2 * p) % HC
                                        nc.tensor.matmul(
                                            pt[:, m * bs + pos:m * bs + pos + cnt],
                                            lhsT=wlr_v[:, 2 * p:2 * p + 2,
                                                       m * HS:(m + 1) * HS],
                                            rhs=_c4(hT_v, ch, 2, a, cnt, st),
                                            start=False, stop=last,
                                            perf_mode=PM.DoubleRow,
                                            skip_group_check=True)
                        else:
                            for m in range(6):
                                for c in range(KC):
                                    runs = seg["l_runs"] if c < HC else seg["r_runs"]
                                    last = c == KC - 1
                                    for (a, st, cnt, pos) in runs:
                                        ch = c % HC
                                        nc.tensor.matmul(
                                            pt[:, m * bs + pos:m * bs + pos + cnt],
                                            lhsT=wlr_v[:, c, m * HS:(m + 1) * HS],
                                            rhs=_c3(hT_v[:, ch, :], a, cnt, st),
                                            start=False, stop=last,
                                            skip_group_check=True)

                        if li >= 3:
                            # warm-PE filler: junk matmuls streamed during
                            # the AllGather wait keep the tensor engine's
                            # p-state high so the real (dependent) matmuls
                            # of the next level run at full clock.  (Reuses
                            # a GEMM psum buffer; safe from L03 on, when the
                            # GEMM -- which itself keeps the PE warm -- is
                            # fully drained.)
                            jt = pgp.tile([128, FTW], F32, tag="pg",
                                          name=f"junk_{li}")
                            for _ in range(40):
                                nc.tensor.matmul(
                                    jt[:, :128], lhsT=ident_sb[:, :],
                                    rhs=ident_sb[:, :],
                                    start=True, stop=True,
                                    skip_group_check=True)
                        if DBG and li == 1:
                            dbg_pt = nc.dram_tensor(f"dbg_pt_{off}",
                                                    [128, 6 * bs], F32,
                                                    kind="ExternalOutput")
                            ptf = ep.tile([HS, 6 * SEG], F32, tag="ptf",
                                          name=f"ptf_{li}")
                            nc.vector.tensor_copy(out=ptf[:, :6 * bs],
                                                  in_=pt[:, :6 * bs])
                            nc.sync.dma_start(out=dbg_pt[:, :],
                                              in_=ptf[:, :6 * bs])
                        # activations: one sigmoid span + one tanh span
                        nc.scalar.activation(
                            g[:, :5 * bs], pt[:, :5 * bs],
                            AF.Sigmoid, scale=1.0 / GSC)
                        nc.scalar.activation(
                            g[:, 5 * bs:6 * bs], pt[:, 5 * bs:6 * bs],
                            AF.Tanh, scale=1.0 / GSC)

                    if DBG and li in (0, 1, 2):
                        dbg = nc.dram_tensor(f"dbg_g{li}_{off}", [HS, 6 * bs],
                                             F32, kind="ExternalOutput")
                        gf = ep.tile([HS, 6 * SEG], F32, tag="gf",
                                     name=f"gf_{li}")
                        nc.vector.tensor_copy(out=gf[:, :6 * bs],
                                              in_=g[:, :6 * bs])
                        nc.sync.dma_start(out=dbg[:, :], in_=gf[:, :6 * bs])

                    def gs(m):
                        return g[:, m * bs:(m + 1) * bs]

                    at = ep.tile([HS, SEG], BF16, tag="ta", name=f"ta_{li}")
                    bt = ep.tile([HS, SEG], BF16, tag="tb", name=f"tb_{li}")

                    # c = ig*u (+ fl*c_l + fr*c_r), written into cT in place
                    nc.vector.tensor_mul(at[:, :bs], gs(G_I), gs(G_U))
                    if li > 0:
                        for (a, st, cnt, pos) in seg["l_runs"]:
                            nc.vector.tensor_mul(
                                _c3(bt, pos, cnt), _c3(g, G_FL * bs + pos, cnt),
                                _c3(cT, a, cnt, st))
                        nc.vector.tensor_add(at[:, :bs], at[:, :bs], bt[:, :bs])
                        for (a, st, cnt, pos) in seg["r_runs"]:
                            nc.vector.tensor_mul(
                                _c3(bt, pos, cnt), _c3(g, G_FR * bs + pos, cnt),
                                _c3(cT, a, cnt, st))
                    # last add writes c into cT in place (and at keeps a copy
                    # for the tanh below via the same op on at)
                    if li > 0:
                        nc.vector.tensor_add(at[:, :bs], at[:, :bs], bt[:, :bs])
                    for (a, st, cnt, pos) in node_runs:
                        nc.vector.tensor_copy(
                            out=_c3(cT, a, cnt, st), in_=_c3(at, pos, cnt))
                    # h = og * tanh(c); hf = rr*(h - px) + px
                    nc.scalar.activation(bt[:, :bs], at[:, :bs], AF.Tanh)
                    nc.vector.tensor_mul(at[:, :bs], gs(G_O), bt[:, :bs])
                    for (a, st, cnt, pos) in node_runs:
                        reg, loc = a // FTW, a % FTW
                        nc.vector.tensor_sub(
                            _c3(bt, pos, cnt), _c3(at, pos, cnt),
                            _c3(pxr[reg], loc, cnt, st))
                    nc.vector.tensor_mul(bt[:, :bs], bt[:, :bs], gs(G_R))
                    for (a, st, cnt, pos) in node_runs:
                        reg, loc = a // FTW, a % FTW
                        # critical path: fp8 stage for the AllGather
                        nc.vector.tensor_add(
                            _c3(stage, off + pos, cnt), _c3(bt, pos, cnt),
                            _c3(pxr[reg], loc, cnt, st))
                    # off critical path: fp32 output rows
                    for (a, st, cnt, pos) in node_runs:
                        reg, loc = a // FTW, a % FTW
                        nc.vector.tensor_add(
                            _c3(houtT, a, cnt, st), _c3(bt, pos, cnt),
                            _c3(pxr[reg], loc, cnt, st))

                if li == nlev - 1:
                    return  # nothing consumes the last level's h
                if DBG and li == 0:
                    dbg_st = nc.dram_tensor("dbg_stage", [HS, max_b],
                                            F32, kind="ExternalOutput")
                    stf = ep.tile([HS, max_b], F32, tag="stf", name="stf")
                    nc.vector.tensor_copy(out=stf[:, :], in_=stage[:, :])
                    nc.sync.dma_start(out=dbg_st[:, :], in_=stf[:, :])
                agi = dp.tile([HS, B], hdt, tag="agin", name=f"agi_{li}")
                ago = dp.tile([H, B], hdt, tag="agout", name=f"ago_{li}",
                              addr_space="Shared")
                nc.sync.dma_start(out=agi[:, :], in_=stage[:, :B])
                nc.gpsimd.collective_compute(
                    "AllGather", ALU.bypass,
                    replica_groups=[list(range(NCORES))],
                    ins=[agi.opt()], outs=[ago.opt()])
                ago_v = ago.rearrange("(c p) b -> p c b", p=HS)
                for seg in L["segs"]:
                    off = seg["off"]
                    for (a, st, cnt, pos) in seg["node_runs"]:
                        src = ago_v[:, :, off + pos: off + pos + cnt]
                        src = src.rearrange("p c (k s) -> p c k s", s=1)
                        if st == 1:
                            dst = hT_v[:, :, a:a + cnt]
                            dst = dst.rearrange("p c (k s) -> p c k s", s=1)
                        else:
                            dst = hT_v[:, :, a:a + st * cnt].rearrange(
                                "p c (k s) -> p c k s", s=st)[:, :, :, 0:1]
                        nc.sync.dma_start(out=dst, in_=src)
                if DBG and li == 0:
                    lo, hi = min(L["nodes"]), max(L["nodes"]) + 1
                    w_ = hi - lo
                    dbg_ht = nc.dram_tensor("dbg_hT", [128, HC * w_],
                                            F32, kind="ExternalOutput")
                    dbg_v = dbg_ht.rearrange("p (c x) -> p c x", c=HC)
                    htf = ep.tile([128, HC * w_], F32, tag="htf", name="htf")
                    htf_v = htf.rearrange("p (c x) -> p c x", c=HC)
                    nc.vector.tensor_copy(out=htf_v[:, :, :],
                                          in_=hT_v[:, :, lo:hi])
                    nc.sync.dma_start(out=dbg_v[:, :, :], in_=htf_v[:, :, :])

            # ---- emission: GEMM regions by first-use level, levels interleaved
            def gemm_upto(ml):
                for (c0, w, lvl) in feat_tiles:
                    if lvl <= ml and c0 not in emitted:
                        emitted.add(c0)
                        with nc.named_scope(f"G{c0:04d}"):
                            emit_gemm_tile(c0, w)

            emitted = set()
            gemm_upto(0)
            with nc.named_scope("L00"):
                emit_level(0)
            gemm_upto(1)
            if nlev > 1:
                with nc.named_scope("L01"):
                    emit_level(1)
            gemm_upto(1 << 30)
            for li in range(2, nlev):
                with nc.named_scope(f"L{li:02d}"):
                    emit_level(li)

            nc.sync.dma_start(out=out[:, :], in_=houtT[:, :])
    return nc


# -------------------------------------------------------------- host logic --

def _prep(inputs, n, f):
    feats = np.asarray(inputs["features"], np.float32)
    wx = np.asarray(inputs["w_ioffux"], np.float32)
    bx = np.asarray(inputs["b_ioffux"], np.float32)
    wl = np.asarray(inputs["w_ioffuh_l"], np.float32)
    bl = np.asarray(inputs["b_ioffuh_l"], np.float32)
    wr = np.asarray(inputs["w_ioffuh_r"], np.float32)
    br = np.asarray(inputs["b_ioffuh_r"], np.float32)
    wpx = np.asarray(inputs["w_px"], np.float32)
    bpx = np.asarray(inputs["b_px"], np.float32)

    featT = np.empty((f + BIAS_ROWS, n), dtype=BF16_NP)
    featT[:f] = feats.T.astype(BF16_NP)
    featT[f:] = np.ones((BIAS_ROWS, n), BF16_NP)
    identm = np.eye(128, dtype=BF16_NP)
    b_all = bx + bl + br

    in_maps = []
    for k in range(NCORES):
        rows = np.concatenate(
            [np.arange(t * H + k * HS, t * H + (k + 1) * HS) for t in GATE_PERM])
        _, hnp = _hdt()
        wlr_T = np.ascontiguousarray(
            np.concatenate([wl[rows], wr[rows]], axis=1).T * GSC).astype(hnp)
        if USE_DR == 2:
            # SwInterleave layout: per (K pair, gate chunk) a 256-col block
            # [A127 B127 A126 B126 ... A0 B0] (pair cols interleaved, reversed)
            X = wlr_T.reshape(KC, 128, 6, 128)[:, :, :, ::-1]
            pr = X.reshape(KC // 2, 2, 128, 6, 128)
            wlr_T = np.ascontiguousarray(
                np.transpose(pr, (2, 0, 3, 4, 1)).reshape(128, -1))
        wxpx = np.concatenate([wx[rows] * GSC, wpx[k * HS:(k + 1) * HS]], axis=0)
        b_aug = np.concatenate([b_all[rows] * GSC, bpx[k * HS:(k + 1) * HS]])
        wxpx_T = np.empty((f + BIAS_ROWS, S6 + HS), dtype=BF16_NP)
        wxpx_T[:f] = wxpx.T.astype(BF16_NP)
        wxpx_T[f:] = np.tile((b_aug / BIAS_ROWS).astype(BF16_NP),
                             (BIAS_ROWS, 1))
        in_maps.append({
            "featT": featT,
            "wlrT": wlr_T,
            "wxpxT": wxpx_T,
            "ident": identm,
        })
    return in_maps


def _assemble(results, n):
    out = np.empty((n, H), np.float32)
    for k in range(NCORES):
        out[:, k * HS:(k + 1) * HS] = results[k]["out"].T
    return out


_CACHE = {}


def _get_nc(inputs):
    feats = np.asarray(inputs["features"])
    n, f = feats.shape
    lc = np.asarray(inputs["left_child"])
    rc = np.asarray(inputs["right_child"])
    key = (n, f, lc.tobytes(), rc.tobytes())
    if key not in _CACHE:
        levels = _schedule(lc, rc, n)
        ftiles = _feat_tiles(levels, n)
        nc = bacc.Bacc(trn_type="TRN2", target_bir_lowering=False,
                       debug=False, num_devices=NCORES)
        build(nc, levels, ftiles, n, f)
        nc.compile()
        _CACHE[key] = nc
    return _CACHE[key], n, f


def kernel(**inputs):
    nc, n, f = _get_nc(inputs)
    in_maps = _prep(inputs, n, f)
    res = run_bass_kernel_spmd(nc, in_maps, core_ids=list(range(NCORES)))
    return _assemble(res.results, n)
